# revision 13
# baseline (speedup 1.0000x reference)
"""Distributed real SHT (spherical harmonic transform) for Trainium2.

Computes, for x [1, 256, 361, 720] f32 and weights [361, 360, 361] f32:
    xf = 2*pi * rfft(x, axis=-1, norm='forward')[..., :361]
    out_re = einsum('bckm,mlk->bclm', Re(xf), weights)
    out_im = einsum('bckm,mlk->bclm', Im(xf), weights)
    return complex64 [1, 256, 360, 361]

Sharding: channels (dim 1) across 8 NeuronCores, 32 channels each.
Per-core two-stage pipeline:
  Stage A: DFT as fp32r matmuls, xf[m, (k,c)] = sum_n F[n, m] * xT[n, (k,c)]
  Stage B: Legendre contraction as bf16 matmuls per m-quad (4x col tiling),
           out[(m',c), l] = sum_k xf[k, (m',c)] * W[m][k, l], l >= 4*(m//4)
The weights are triangular (zero for l < m); only the l >= 4*floor(m/4)
blocks are shipped/multiplied, and the l < m region of the output is exactly
zero (restored host-side).
"""

import numpy as np
import ml_dtypes

NLAT = 361          # latitudes (k)
NLON = 720          # longitudes (n)
LMAX = 360          # output degree count (l = 0..359)
MMAX = 361          # rfft modes kept (m = 0..360); m=360 output is all-zero
C = 256
N_CORES = 8
C_LOC = C // N_CORES        # 32 channels per core
KPAD = 384                  # latitude padded to 3*128
MPAD = 384                  # modes padded to 3*128
NCH = 6                     # n (720) split into 6 chunks of 120
NW = NLON // NCH            # 120
KCG = 6                     # (k,c) columns split into 6 groups of 2048 (64 k each)
GW = KPAD * C_LOC // KCG    # 2048 columns per group
NQ = LMAX // 4              # 90 m-quads (m = 4q + m', m' in 0..3)

BF16 = ml_dtypes.bfloat16

# quad l-block sizes and blob offsets (static)
_QL = [LMAX - 4 * q for q in range(NQ)]
_WQ_OFF = np.cumsum([0] + [3 * 128 * 4 * L for L in _QL]).tolist()
_OB_OFF = np.cumsum([0] + [2 * 128 * L for L in _QL]).tolist()
WQ_TOTAL = _WQ_OFF[-1]
OB_TOTAL = _OB_OFF[-1]

_CACHE = {}


def _build_bass(reps=1):
    import concourse.bass as bass
    import concourse.mybir as mybir
    import concourse.tile as tile
    from concourse import bacc
    from contextlib import nullcontext

    f32r = mybir.dt.float32r
    bf16 = mybir.dt.bfloat16
    f32 = mybir.dt.float32

    nc = bacc.Bacc("TRN2", target_bir_lowering=False, debug=False,
                   num_devices=N_CORES)

    xt_d = nc.dram_tensor("xt", [NLON, KPAD * C_LOC], f32r, kind="ExternalInput")
    f_d = nc.dram_tensor("fm", [NLON, 2 * MPAD], f32r, kind="ExternalInput")
    wq_d = nc.dram_tensor("wq", [WQ_TOTAL], bf16, kind="ExternalInput")
    ob_d = nc.dram_tensor("ob", [OB_TOTAL], f32, kind="ExternalOutput")

    with tile.TileContext(nc) as tc:
        with (
            tc.tile_pool(name="dram", bufs=1, space="DRAM") as dram,
            tc.tile_pool(name="fpool", bufs=1) as fpool,
            tc.tile_pool(name="xtp", bufs=2) as xtp,
            tc.tile_pool(name="evict", bufs=4) as evp,
            tc.tile_pool(name="xfl", bufs=6) as xflp,
            tc.tile_pool(name="wt", bufs=6) as wtp,
            tc.tile_pool(name="outp", bufs=4) as outp,
            tc.tile_pool(name="psA", bufs=4, space="PSUM") as psA,
            tc.tile_pool(name="psB", bufs=4, space="PSUM") as psB,
            tc.For_i(0, reps, 1) if reps > 1 else nullcontext(),
        ):
            # intermediate xf in DRAM: [kc 3, kk 128, m MPAD, ri 2, c 32] bf16
            xf_t = dram.tile([3, 128, MPAD, 2, C_LOC], bf16)

            # F matrix resident in SBUF: [120, nchunk 6, (ri 2, m 384)]
            f_tile = fpool.tile([NW, NCH, 2 * MPAD], f32r)
            nc.sync.dma_start(
                f_tile[:],
                f_d[:].rearrange("(a p) f -> p a f", p=NW),
            )

            # ---------------- Stage A: DFT over longitude ----------------
            for g in range(KCG):
                xt_tile = xtp.tile([NW, NCH * GW], f32r, tag="xt")
                nc.sync.dma_start(
                    xt_tile[:].rearrange("p (a q) -> p a q", a=NCH),
                    xt_d[:, g * GW:(g + 1) * GW].rearrange(
                        "(a p) q -> p a q", p=NW),
                )
                kc_g = (g * 64) // 128          # which 128-row k chunk
                kk0 = (g * 64) % 128            # offset inside it
                for mch in range(3):
                    for ri in range(2):
                        ev = evp.tile([128, GW], bf16, tag="ev")
                        for ch in range(4):
                            ps = psA.tile([128, 512], f32, tag="psA")
                            for ncc in range(NCH):
                                col0 = ri * MPAD + mch * 128
                                nc.tensor.matmul(
                                    ps[:],
                                    f_tile[:, ncc, col0:col0 + 128],
                                    xt_tile[:, ncc * GW + ch * 512:
                                            ncc * GW + (ch + 1) * 512],
                                    start=(ncc == 0),
                                    stop=(ncc == NCH - 1),
                                )
                            nc.vector.tensor_copy(
                                ev[:, ch * 512:(ch + 1) * 512], ps[:])
                        # ev is [m 128, (k 64, c 32)] -> xf[kc_g, kk0:+64, mch*128:+128, ri, :]
                        nc.sync.dma_start(
                            xf_t[kc_g, kk0:kk0 + 64,
                                 mch * 128:(mch + 1) * 128, ri, :]
                            .rearrange("k m c -> m k c"),
                            ev[:].rearrange("m (k c) -> m k c", k=64),
                        )

            # ---------------- Stage B: Legendre contraction ----------------
            for q in range(NQ):
                L = _QL[q]
                lts = []
                wts = []
                for kc in range(3):
                    lt = xflp.tile([128, 256], bf16, tag="lt")
                    nc.sync.dma_start(
                        lt[:].rearrange("p (a r c) -> p a r c", a=4, r=2),
                        xf_t[kc, :, 4 * q:4 * q + 4, :, :],
                    )
                    lts.append(lt)
                    wt = wtp.tile([128, 4 * L], bf16, tag="wt")
                    off = _WQ_OFF[q] + kc * 128 * 4 * L
                    nc.sync.dma_start(
                        wt[:],
                        wq_d[off:off + 128 * 4 * L].rearrange(
                            "(p f) -> p f", p=128),
                    )
                    wts.append(wt)
                for ri in range(2):
                    ps = psB.tile([128, L], f32, tag="psB")
                    for mp in range(4):
                        for kc in range(3):
                            nc.tensor.matmul(
                                ps[mp * 32:(mp + 1) * 32, :],
                                lts[kc][:, mp * 64 + ri * 32:
                                        mp * 64 + ri * 32 + 32],
                                wts[kc][:, mp * L:(mp + 1) * L],
                                start=(kc == 0),
                                stop=(kc == 2),
                                tile_position=(0, mp * 32),
                            )
                    ot = outp.tile([128, L], f32, tag="ot")
                    nc.vector.tensor_copy(ot[:], ps[:])
                    off = _OB_OFF[q] + ri * 128 * L
                    nc.sync.dma_start(
                        ob_d[off:off + 128 * L].rearrange("(p f) -> p f", p=128),
                        ot[:],
                    )

    nc.compile()
    return nc


def _dft_matrix():
    n = np.arange(NLON, dtype=np.float64)[:, None]
    m = np.arange(MMAX, dtype=np.float64)[None, :]
    ang = 2.0 * np.pi * n * m / NLON
    coef = 2.0 * np.pi / NLON
    F = np.zeros((NLON, 2, MPAD), dtype=np.float32)
    F[:, 0, :MMAX] = (coef * np.cos(ang)).astype(np.float32)
    F[:, 1, :MMAX] = (-coef * np.sin(ang)).astype(np.float32)
    return F.reshape(NLON, 2 * MPAD)


def _pack_weights(weights):
    # weights [MMAX, LMAX, NLAT] f32 -> per-quad blob bf16
    Wt = np.ascontiguousarray(np.transpose(weights, (0, 2, 1)))  # [m, k, l]
    blob = np.empty(WQ_TOTAL, dtype=BF16)
    for q in range(NQ):
        L = _QL[q]
        sub = np.zeros((4, KPAD, L), dtype=np.float32)
        m_hi = min(4 * q + 4, MMAX)
        sub[:m_hi - 4 * q, :NLAT, :] = Wt[4 * q:m_hi, :, 4 * q:]
        # [4 m', KPAD k, L] -> [kc 3, kk 128, m' 4, L]
        arr = sub.reshape(4, 3, 128, L).transpose(1, 2, 0, 3)
        blob[_WQ_OFF[q]:_WQ_OFF[q + 1]] = arr.astype(BF16).ravel()
    return blob


class _Runner:
    """jit(shard_map(bass_exec)) over the 8 cores, inputs stay addressable
    as sharded jax arrays so repeated timed executions skip host transfer."""

    def __init__(self, nc):
        import jax
        import concourse.mybir as mybir
        from jax.experimental.shard_map import shard_map
        from jax.sharding import Mesh, PartitionSpec, NamedSharding
        from concourse.bass2jax import (
            _bass_exec_p, install_neuronx_cc_hook, partition_id_tensor)

        install_neuronx_cc_hook()
        self.jax = jax
        self.nc = nc
        part_name = (nc.partition_id_tensor.name
                     if nc.partition_id_tensor else None)
        in_names, out_names, out_avals, zero_outs = [], [], [], []
        for alloc in nc.m.functions[0].allocations:
            if not isinstance(alloc, mybir.MemoryLocationSet):
                continue
            name = alloc.memorylocations[0].name
            if alloc.kind == "ExternalInput":
                if name != part_name:
                    in_names.append(name)
            elif alloc.kind == "ExternalOutput":
                shape = tuple(alloc.tensor_shape)
                dtype = mybir.dt.np(alloc.dtype)
                out_names.append(name)
                out_avals.append(jax.core.ShapedArray(shape, dtype))
                zero_outs.append(np.zeros(shape, dtype))
        self.in_names = list(in_names)
        self.out_names = out_names
        self.out_avals = out_avals
        self.zero_outs = zero_outs
        all_names = in_names + out_names
        if part_name is not None:
            all_names = all_names + [part_name]

        def _body(*args):
            operands = list(args)
            if part_name is not None:
                operands.append(partition_id_tensor())
            outs = _bass_exec_p.bind(
                *operands,
                out_avals=tuple(out_avals),
                in_names=tuple(all_names),
                out_names=tuple(out_names),
                lowering_input_output_aliases=(),
                sim_require_finite=True,
                sim_require_nnan=True,
                nc=nc,
            )
            return tuple(outs)

        devices = jax.devices()[:N_CORES]
        mesh = Mesh(np.asarray(devices), ("core",))
        spec = PartitionSpec("core")
        n_args = len(in_names) + len(out_names)
        self.sharding = NamedSharding(mesh, spec)
        self.fn = jax.jit(
            shard_map(_body, mesh=mesh,
                      in_specs=(spec,) * n_args,
                      out_specs=(spec,) * len(out_names),
                      check_rep=False),
            keep_unused=True,
        )

    def make_chain_fn(self, n_chain):
        """Build a jitted fn that executes the NEFF n_chain times serially
        (each iteration's first output feeds the next iteration's output
        placeholder, creating a data dependency that defeats CSE).  Used to
        measure per-execution device time above the fixed dispatch floor."""
        import jax
        from jax.experimental.shard_map import shard_map
        from jax.sharding import Mesh, PartitionSpec
        from concourse.bass2jax import _bass_exec_p, partition_id_tensor

        nc = self.nc
        part_name = (nc.partition_id_tensor.name
                     if nc.partition_id_tensor else None)
        all_names = list(self.in_names) + list(self.out_names)
        if part_name is not None:
            all_names = all_names + [part_name]
        out_avals = self.out_avals
        out_names = self.out_names

        def _body(*args):
            operands = list(args)
            if part_name is not None:
                operands.append(partition_id_tensor())
            last = None
            for _ in range(n_chain):
                # bass_exec carries BassEffect, so repeated identical calls
                # are neither CSE'd nor DCE'd; they serialize on the device
                # stream.
                last = _bass_exec_p.bind(
                    *operands,
                    out_avals=tuple(out_avals),
                    in_names=tuple(all_names),
                    out_names=tuple(out_names),
                    lowering_input_output_aliases=(),
                    sim_require_finite=True,
                    sim_require_nnan=True,
                    nc=nc,
                )
            return tuple(last)

        devices = self.jax.devices()[:N_CORES]
        mesh = Mesh(np.asarray(devices), ("core",))
        spec = PartitionSpec("core")
        n_args = len(self.in_names) + len(self.out_names)
        return jax.jit(
            shard_map(_body, mesh=mesh,
                      in_specs=(spec,) * n_args,
                      out_specs=(spec,) * len(self.out_names),
                      check_rep=False),
            keep_unused=True,
        )

    def device_args(self, in_maps):
        """Concat per-core inputs on axis 0 and put on the mesh."""
        jax = self.jax
        args = []
        for i, name in enumerate(self.in_names):
            cat = np.concatenate([m[name] for m in in_maps], axis=0)
            args.append(jax.device_put(cat, self.sharding))
        for z in self.zero_outs:
            cat = np.zeros((N_CORES * z.shape[0], *z.shape[1:]), z.dtype)
            args.append(jax.device_put(cat, self.sharding))
        return args

    def execute(self, args):
        outs = self.fn(*args)
        self.jax.block_until_ready(outs)
        return outs

    def run(self, in_maps):
        outs = self.execute(self.device_args(in_maps))
        results = []
        for c in range(N_CORES):
            r = {}
            for i, name in enumerate(self.out_names):
                full = np.asarray(outs[i])
                r[name] = full.reshape(N_CORES, *self.out_avals[i].shape)[c]
            results.append(r)
        return results


def get_runner(reps=1):
    key = ("runner", reps)
    if key not in _CACHE:
        _CACHE[key] = _Runner(_build_bass(reps))
    if "F" not in _CACHE:
        _CACHE["F"] = _dft_matrix()
    return _CACHE[key]


def prepare_in_maps(x, weights):
    if "F" not in _CACHE:
        _CACHE["F"] = _dft_matrix()
    F = _CACHE["F"]
    x = np.asarray(x, dtype=np.float32)
    weights = np.asarray(weights, dtype=np.float32)
    wq = _pack_weights(weights)
    in_maps = []
    for p in range(N_CORES):
        xs = x[0, p * C_LOC:(p + 1) * C_LOC]          # [32, 361, 720]
        xt = np.zeros((NLON, KPAD, C_LOC), dtype=np.float32)
        xt[:, :NLAT, :] = xs.transpose(2, 1, 0)
        in_maps.append({
            "xt": np.ascontiguousarray(xt.reshape(NLON, KPAD * C_LOC)),
            "fm": F,
            "wq": wq,
        })
    return in_maps


def unpack_results(results):
    out_re = np.zeros((C, LMAX, MMAX), dtype=np.float32)
    out_im = np.zeros((C, LMAX, MMAX), dtype=np.float32)
    for p in range(N_CORES):
        ob = results[p]["ob"]
        c0 = p * C_LOC
        for q in range(NQ):
            L = _QL[q]
            arr = ob[_OB_OFF[q]:_OB_OFF[q + 1]].reshape(2, 4, C_LOC, L)
            # arr[ri, m', c, l'] -> out[c, l0+l', 4q+m']
            out_re[c0:c0 + C_LOC, 4 * q:, 4 * q:4 * q + 4] = \
                arr[0].transpose(1, 2, 0)
            out_im[c0:c0 + C_LOC, 4 * q:, 4 * q:4 * q + 4] = \
                arr[1].transpose(1, 2, 0)

    out = (out_re + 1j * out_im).astype(np.complex64)
    return out.reshape(1, C, LMAX, MMAX)


def kernel(x, weights):
    runner = get_runner()
    in_maps = prepare_in_maps(x, weights)
    results = runner.run(in_maps)
    return unpack_results(results)


# revision 19
# speedup vs baseline: 1.1752x; 1.1752x over previous
"""Distributed real SHT (spherical harmonic transform) for Trainium2.

Computes, for x [1, 256, 361, 720] f32 and weights [361, 360, 361] f32:
    xf = 2*pi * rfft(x, axis=-1, norm='forward')[..., :361]
    out_re = einsum('bckm,mlk->bclm', Re(xf), weights)
    out_im = einsum('bckm,mlk->bclm', Im(xf), weights)
    return complex64 [1, 256, 360, 361]

Sharding: channels (dim 1) across 8 NeuronCores, 32 channels each.
Per-core two-stage pipeline:
  Stage A: DFT as fp32r matmuls, xf[m, (k,c)] = sum_n F[n, m] * xT[n, (k,c)]
  Stage B: Legendre contraction as bf16 matmuls per m-quad (4x col tiling),
           out[(m',c), l] = sum_k xf[k, (m',c)] * W[m][k, l], l >= 4*(m//4)
The weights are triangular (zero for l < m); only the l >= 4*floor(m/4)
blocks are shipped/multiplied, and the l < m region of the output is exactly
zero (restored host-side).
"""

import numpy as np
import ml_dtypes

NLAT = 361          # latitudes (k)
NLON = 720          # longitudes (n)
LMAX = 360          # output degree count (l = 0..359)
MMAX = 361          # rfft modes kept (m = 0..360); m=360 output is all-zero
C = 256
N_CORES = 8
C_LOC = C // N_CORES        # 32 channels per core
KPAD = 384                  # latitude padded to 3*128
MPAD = 384                  # modes padded to 3*128
NCH = 6                     # n (720) split into 6 chunks of 120
NW = NLON // NCH            # 120
KCG = 6                     # (k,c) columns split into 6 groups of 2048 (64 k each)
GW = KPAD * C_LOC // KCG    # 2048 columns per group
NQ = LMAX // 4              # 90 m-quads (m = 4q + m', m' in 0..3)

BF16 = ml_dtypes.bfloat16

# quad l-block sizes and blob offsets (static)
_QL = [LMAX - 4 * q for q in range(NQ)]
_WQ_OFF = np.cumsum([0] + [3 * 128 * 4 * L for L in _QL]).tolist()
_OB_OFF = np.cumsum([0] + [2 * 128 * L for L in _QL]).tolist()
WQ_TOTAL = _WQ_OFF[-1]
OB_TOTAL = _OB_OFF[-1]

_CACHE = {}


def _build_bass(reps=1, stage="AB"):
    import concourse.bass as bass
    import concourse.mybir as mybir
    import concourse.tile as tile
    from concourse import bacc
    from contextlib import nullcontext

    f32r = mybir.dt.float32r
    bf16 = mybir.dt.bfloat16
    f32 = mybir.dt.float32

    nc = bacc.Bacc("TRN2", target_bir_lowering=False, debug=False,
                   num_devices=N_CORES)

    xt_d = nc.dram_tensor("xt", [NLON, KPAD * C_LOC], f32r, kind="ExternalInput")
    f_d = nc.dram_tensor("fm", [NLON, 2 * MPAD], f32r, kind="ExternalInput")
    wq_d = nc.dram_tensor("wq", [WQ_TOTAL], bf16, kind="ExternalInput")
    ob_d = nc.dram_tensor("ob", [OB_TOTAL], f32, kind="ExternalOutput")

    with tile.TileContext(nc) as tc:
        with (
            tc.tile_pool(name="dram", bufs=1, space="DRAM") as dram,
            tc.tile_pool(name="fpool", bufs=1) as fpool,
            tc.tile_pool(name="xtp", bufs=2) as xtp,
            tc.tile_pool(name="evict", bufs=2) as evp,
            tc.tile_pool(name="xfl", bufs=4) as xflp,
            tc.tile_pool(name="wt", bufs=3) as wtp,
            tc.tile_pool(name="outp", bufs=3) as outp,
            tc.tile_pool(name="psA", bufs=4, space="PSUM") as psA,
            tc.tile_pool(name="psB", bufs=4, space="PSUM") as psB,
            tc.For_i(0, reps, 1) if reps > 1 else nullcontext(),
        ):
            # intermediate xf in DRAM, one tensor per m-chunk of 128:
            # [kc 3, kk 128, m 128, ri 2, c 32] bf16 (per-mch split lets
            # stage B start before all of stage A has finished)
            xf_ms = [dram.tile([3, 128, 128, 2, C_LOC], bf16, name=f"xfm{i}", tag=f"xfm{i}")
                     for i in range(3)]

            # F matrix resident in SBUF: [120, nchunk 6, (ri 2, m 384)]
            f_tile = fpool.tile([NW, NCH, 2 * MPAD], f32r)
            nc.sync.dma_start(
                f_tile[:],
                f_d[:].rearrange("(a p) f -> p a f", p=NW),
            )

            # ---------------- Stage A: DFT over longitude ----------------
            for g in range(KCG if stage in ("AB", "A") else 0):
                xt_tile = xtp.tile([NW, NCH * GW], f32r, tag="xt")
                nc.sync.dma_start(
                    xt_tile[:].rearrange("p (a q) -> p a q", a=NCH),
                    xt_d[:, g * GW:(g + 1) * GW].rearrange(
                        "(a p) q -> p a q", p=NW),
                )
                kc_g = (g * 64) // 128          # which 128-row k chunk
                kk0 = (g * 64) % 128            # offset inside it
                for mch in range(3):
                    # ev holds both re and im interleaved: [m 128, (k 64, ri 2, c 32)]
                    ev = evp.tile([128, 2 * GW], bf16, tag="ev")
                    evv = ev[:].rearrange("m (k r c) -> m k r c", k=64, r=2)
                    for ri in range(2):
                        for ch in range(4):
                            ps = psA.tile([128, 512], f32, tag="psA")
                            for ncc in range(NCH):
                                col0 = ri * MPAD + mch * 128
                                nc.tensor.matmul(
                                    ps[:],
                                    f_tile[:, ncc, col0:col0 + 128],
                                    xt_tile[:, ncc * GW + ch * 512:
                                            ncc * GW + (ch + 1) * 512],
                                    start=(ncc == 0),
                                    stop=(ncc == NCH - 1),
                                )
                            nc.vector.tensor_copy(
                                evv[:, ch * 16:(ch + 1) * 16, ri, :],
                                ps[:].rearrange("m (k c) -> m k c", k=16))
                    nc.scalar.dma_start(
                        xf_ms[mch][kc_g, kk0:kk0 + 64, :, :, :]
                        .rearrange("k m r c -> m k r c"),
                        evv,
                    )

            # ---------------- Stage B: Legendre contraction ----------------
            # octets of 8 quads (32 m) share one xf load per kc
            n_oct = (NQ + 7) // 8
            for o in range(n_oct if stage in ("AB", "B") else 0):
                q0 = 8 * o
                q1 = min(q0 + 8, NQ)
                nq_o = q1 - q0
                mch = (4 * q0) // 128
                m0 = 4 * q0 - 128 * mch
                lts = []
                for kc in range(3):
                    lt = xflp.tile([128, nq_o * 256], bf16, tag="lt")
                    nc.sync.dma_start(
                        lt[:].rearrange("p (a r c) -> p a r c",
                                        a=4 * nq_o, r=2),
                        xf_ms[mch][kc, :, m0:m0 + 4 * nq_o, :, :],
                    )
                    lts.append(lt)
                for q in range(q0, q1):
                    L = _QL[q]
                    ql = q - q0
                    wt = wtp.tile([128, 3 * 4 * L], bf16, tag="wt")
                    off = _WQ_OFF[q]
                    nc.sync.dma_start(
                        wt[:].rearrange("p (kc f) -> p kc f", kc=3),
                        wq_d[off:off + 3 * 128 * 4 * L].rearrange(
                            "(kc p f) -> p kc f", kc=3, p=128),
                    )
                    ot = outp.tile([128, 2 * L], f32, tag="ot")
                    for ri in range(2):
                        ps = psB.tile([128, L], f32, tag="psB")
                        for mp in range(4):
                            for kc in range(3):
                                nc.tensor.matmul(
                                    ps[mp * 32:(mp + 1) * 32, :],
                                    lts[kc][:, (ql * 4 + mp) * 64 + ri * 32:
                                            (ql * 4 + mp) * 64 + ri * 32 + 32],
                                    wt[:, (kc * 4 + mp) * L:
                                       (kc * 4 + mp) * L + L],
                                    start=(kc == 0),
                                    stop=(kc == 2),
                                    tile_position=(0, mp * 32),
                                )
                        nc.vector.tensor_copy(
                            ot[:, ri * L:(ri + 1) * L], ps[:])
                    off = _OB_OFF[q]
                    nc.scalar.dma_start(
                        ob_d[off:off + 2 * 128 * L].rearrange(
                            "(r p l) -> p r l", r=2, p=128),
                        ot[:].rearrange("p (r l) -> p r l", r=2),
                    )

    nc.compile()
    return nc


def _dft_matrix():
    n = np.arange(NLON, dtype=np.float64)[:, None]
    m = np.arange(MMAX, dtype=np.float64)[None, :]
    ang = 2.0 * np.pi * n * m / NLON
    coef = 2.0 * np.pi / NLON
    F = np.zeros((NLON, 2, MPAD), dtype=np.float32)
    F[:, 0, :MMAX] = (coef * np.cos(ang)).astype(np.float32)
    F[:, 1, :MMAX] = (-coef * np.sin(ang)).astype(np.float32)
    return F.reshape(NLON, 2 * MPAD)


def _pack_weights(weights):
    # weights [MMAX, LMAX, NLAT] f32 -> per-quad blob bf16
    Wt = np.ascontiguousarray(np.transpose(weights, (0, 2, 1)))  # [m, k, l]
    blob = np.empty(WQ_TOTAL, dtype=BF16)
    for q in range(NQ):
        L = _QL[q]
        sub = np.zeros((4, KPAD, L), dtype=np.float32)
        m_hi = min(4 * q + 4, MMAX)
        sub[:m_hi - 4 * q, :NLAT, :] = Wt[4 * q:m_hi, :, 4 * q:]
        # [4 m', KPAD k, L] -> [kc 3, kk 128, m' 4, L]
        arr = sub.reshape(4, 3, 128, L).transpose(1, 2, 0, 3)
        blob[_WQ_OFF[q]:_WQ_OFF[q + 1]] = arr.astype(BF16).ravel()
    return blob


class _Runner:
    """jit(shard_map(bass_exec)) over the 8 cores, inputs stay addressable
    as sharded jax arrays so repeated timed executions skip host transfer."""

    def __init__(self, nc):
        import jax
        import concourse.mybir as mybir
        from jax.experimental.shard_map import shard_map
        from jax.sharding import Mesh, PartitionSpec, NamedSharding
        from concourse.bass2jax import (
            _bass_exec_p, install_neuronx_cc_hook, partition_id_tensor)

        install_neuronx_cc_hook()
        self.jax = jax
        self.nc = nc
        part_name = (nc.partition_id_tensor.name
                     if nc.partition_id_tensor else None)
        in_names, out_names, out_avals, zero_outs = [], [], [], []
        for alloc in nc.m.functions[0].allocations:
            if not isinstance(alloc, mybir.MemoryLocationSet):
                continue
            name = alloc.memorylocations[0].name
            if alloc.kind == "ExternalInput":
                if name != part_name:
                    in_names.append(name)
            elif alloc.kind == "ExternalOutput":
                shape = tuple(alloc.tensor_shape)
                dtype = mybir.dt.np(alloc.dtype)
                out_names.append(name)
                out_avals.append(jax.core.ShapedArray(shape, dtype))
                zero_outs.append(np.zeros(shape, dtype))
        self.in_names = list(in_names)
        self.out_names = out_names
        self.out_avals = out_avals
        self.zero_outs = zero_outs
        all_names = in_names + out_names
        if part_name is not None:
            all_names = all_names + [part_name]

        def _body(*args):
            operands = list(args)
            if part_name is not None:
                operands.append(partition_id_tensor())
            outs = _bass_exec_p.bind(
                *operands,
                out_avals=tuple(out_avals),
                in_names=tuple(all_names),
                out_names=tuple(out_names),
                lowering_input_output_aliases=(),
                sim_require_finite=True,
                sim_require_nnan=True,
                nc=nc,
            )
            return tuple(outs)

        devices = jax.devices()[:N_CORES]
        mesh = Mesh(np.asarray(devices), ("core",))
        spec = PartitionSpec("core")
        n_args = len(in_names) + len(out_names)
        self.sharding = NamedSharding(mesh, spec)
        self.fn = jax.jit(
            shard_map(_body, mesh=mesh,
                      in_specs=(spec,) * n_args,
                      out_specs=(spec,) * len(out_names),
                      check_rep=False),
            keep_unused=True,
        )

    def make_chain_fn(self, n_chain):
        """Build a jitted fn that executes the NEFF n_chain times serially
        (each iteration's first output feeds the next iteration's output
        placeholder, creating a data dependency that defeats CSE).  Used to
        measure per-execution device time above the fixed dispatch floor."""
        import jax
        from jax.experimental.shard_map import shard_map
        from jax.sharding import Mesh, PartitionSpec
        from concourse.bass2jax import _bass_exec_p, partition_id_tensor

        nc = self.nc
        part_name = (nc.partition_id_tensor.name
                     if nc.partition_id_tensor else None)
        all_names = list(self.in_names) + list(self.out_names)
        if part_name is not None:
            all_names = all_names + [part_name]
        out_avals = self.out_avals
        out_names = self.out_names

        def _body(*args):
            operands = list(args)
            if part_name is not None:
                operands.append(partition_id_tensor())
            last = None
            for _ in range(n_chain):
                # bass_exec carries BassEffect, so repeated identical calls
                # are neither CSE'd nor DCE'd; they serialize on the device
                # stream.
                last = _bass_exec_p.bind(
                    *operands,
                    out_avals=tuple(out_avals),
                    in_names=tuple(all_names),
                    out_names=tuple(out_names),
                    lowering_input_output_aliases=(),
                    sim_require_finite=True,
                    sim_require_nnan=True,
                    nc=nc,
                )
            return tuple(last)

        devices = self.jax.devices()[:N_CORES]
        mesh = Mesh(np.asarray(devices), ("core",))
        spec = PartitionSpec("core")
        n_args = len(self.in_names) + len(self.out_names)
        return jax.jit(
            shard_map(_body, mesh=mesh,
                      in_specs=(spec,) * n_args,
                      out_specs=(spec,) * len(self.out_names),
                      check_rep=False),
            keep_unused=True,
        )

    def device_args(self, in_maps):
        """Concat per-core inputs on axis 0 and put on the mesh."""
        jax = self.jax
        args = []
        for i, name in enumerate(self.in_names):
            cat = np.concatenate([m[name] for m in in_maps], axis=0)
            args.append(jax.device_put(cat, self.sharding))
        for z in self.zero_outs:
            cat = np.zeros((N_CORES * z.shape[0], *z.shape[1:]), z.dtype)
            args.append(jax.device_put(cat, self.sharding))
        return args

    def execute(self, args):
        outs = self.fn(*args)
        self.jax.block_until_ready(outs)
        return outs

    def run(self, in_maps):
        outs = self.execute(self.device_args(in_maps))
        results = []
        for c in range(N_CORES):
            r = {}
            for i, name in enumerate(self.out_names):
                full = np.asarray(outs[i])
                r[name] = full.reshape(N_CORES, *self.out_avals[i].shape)[c]
            results.append(r)
        return results


def get_runner(reps=1, stage="AB"):
    key = ("runner", reps, stage)
    if key not in _CACHE:
        _CACHE[key] = _Runner(_build_bass(reps, stage))
    if "F" not in _CACHE:
        _CACHE["F"] = _dft_matrix()
    return _CACHE[key]


def prepare_in_maps(x, weights):
    if "F" not in _CACHE:
        _CACHE["F"] = _dft_matrix()
    F = _CACHE["F"]
    x = np.asarray(x, dtype=np.float32)
    weights = np.asarray(weights, dtype=np.float32)
    wq = _pack_weights(weights)
    in_maps = []
    for p in range(N_CORES):
        xs = x[0, p * C_LOC:(p + 1) * C_LOC]          # [32, 361, 720]
        xt = np.zeros((NLON, KPAD, C_LOC), dtype=np.float32)
        xt[:, :NLAT, :] = xs.transpose(2, 1, 0)
        in_maps.append({
            "xt": np.ascontiguousarray(xt.reshape(NLON, KPAD * C_LOC)),
            "fm": F,
            "wq": wq,
        })
    return in_maps


def unpack_results(results):
    out_re = np.zeros((C, LMAX, MMAX), dtype=np.float32)
    out_im = np.zeros((C, LMAX, MMAX), dtype=np.float32)
    for p in range(N_CORES):
        ob = results[p]["ob"]
        c0 = p * C_LOC
        for q in range(NQ):
            L = _QL[q]
            arr = ob[_OB_OFF[q]:_OB_OFF[q + 1]].reshape(2, 4, C_LOC, L)
            # arr[ri, m', c, l'] -> out[c, l0+l', 4q+m']
            out_re[c0:c0 + C_LOC, 4 * q:, 4 * q:4 * q + 4] = \
                arr[0].transpose(1, 2, 0)
            out_im[c0:c0 + C_LOC, 4 * q:, 4 * q:4 * q + 4] = \
                arr[1].transpose(1, 2, 0)

    out = (out_re + 1j * out_im).astype(np.complex64)
    return out.reshape(1, C, LMAX, MMAX)


def kernel(x, weights):
    runner = get_runner()
    in_maps = prepare_in_maps(x, weights)
    results = runner.run(in_maps)
    return unpack_results(results)


# revision 20
# speedup vs baseline: 1.4317x; 1.2182x over previous
"""Distributed real SHT (spherical harmonic transform) for Trainium2.

Computes, for x [1, 256, 361, 720] f32 and weights [361, 360, 361] f32:
    xf = 2*pi * rfft(x, axis=-1, norm='forward')[..., :361]
    out_re = einsum('bckm,mlk->bclm', Re(xf), weights)
    out_im = einsum('bckm,mlk->bclm', Im(xf), weights)
    return complex64 [1, 256, 360, 361]

Sharding: channels (dim 1) across 8 NeuronCores, 32 channels each.
Per-core two-stage pipeline:
  Stage A: DFT as fp32r matmuls, xf[m, (k,c)] = sum_n F[n, m] * xT[n, (k,c)]
  Stage B: Legendre contraction as bf16 matmuls per m-quad (4x col tiling),
           out[(m',c), l] = sum_k xf[k, (m',c)] * W[m][k, l], l >= 4*(m//4)
The weights are triangular (zero for l < m); only the l >= 4*floor(m/4)
blocks are shipped/multiplied, and the l < m region of the output is exactly
zero (restored host-side).
"""

import numpy as np
import ml_dtypes

NLAT = 361          # latitudes (k)
NLON = 720          # longitudes (n)
LMAX = 360          # output degree count (l = 0..359)
MMAX = 361          # rfft modes kept (m = 0..360); m=360 output is all-zero
C = 256
N_CORES = 8
C_LOC = C // N_CORES        # 32 channels per core
KPAD = 384                  # latitude padded to 3*128
MPAD = 384                  # modes padded to 3*128
NPAD = 384                  # folded longitude (361 rows) padded to 3*128
NCH = 3                     # folded n split into 3 chunks of 128
NW = 128
KCG = 6                     # (k,c) columns split into 6 groups of 2048 (64 k each)
GW = KPAD * C_LOC // KCG    # 2048 columns per group
NQ = LMAX // 4              # 90 m-quads (m = 4q + m', m' in 0..3)

BF16 = ml_dtypes.bfloat16

# quad l-block sizes and blob offsets (static)
_QL = [LMAX - 4 * q for q in range(NQ)]
_WQ_OFF = np.cumsum([0] + [3 * 128 * 4 * L for L in _QL]).tolist()
_OB_OFF = np.cumsum([0] + [2 * 128 * L for L in _QL]).tolist()
WQ_TOTAL = _WQ_OFF[-1]
OB_TOTAL = _OB_OFF[-1]

_CACHE = {}


def _build_bass(reps=1, stage="AB"):
    import concourse.bass as bass
    import concourse.mybir as mybir
    import concourse.tile as tile
    from concourse import bacc
    from contextlib import nullcontext

    f32r = mybir.dt.float32r
    bf16 = mybir.dt.bfloat16
    f32 = mybir.dt.float32

    nc = bacc.Bacc("TRN2", target_bir_lowering=False, debug=False,
                   num_devices=N_CORES)

    # xt rows: (par 2, npad 384) folded longitudes; par 0 = cos branch
    # (x[n] + x[720-n]), par 1 = sin branch (x[n] - x[720-n])
    xt_d = nc.dram_tensor("xt", [2 * NPAD, KPAD * C_LOC], f32r,
                          kind="ExternalInput")
    f_d = nc.dram_tensor("fm", [NPAD, 2 * MPAD], f32r, kind="ExternalInput")
    wq_d = nc.dram_tensor("wq", [WQ_TOTAL], bf16, kind="ExternalInput")
    ob_d = nc.dram_tensor("ob", [OB_TOTAL], f32, kind="ExternalOutput")

    with tile.TileContext(nc) as tc:
        with (
            tc.tile_pool(name="dram", bufs=1, space="DRAM") as dram,
            tc.tile_pool(name="fpool", bufs=1) as fpool,
            tc.tile_pool(name="xtp", bufs=2) as xtp,
            tc.tile_pool(name="evict", bufs=2) as evp,
            tc.tile_pool(name="xfl", bufs=4) as xflp,
            tc.tile_pool(name="wt", bufs=3) as wtp,
            tc.tile_pool(name="outp", bufs=3) as outp,
            tc.tile_pool(name="psA", bufs=4, space="PSUM") as psA,
            tc.tile_pool(name="psB", bufs=4, space="PSUM") as psB,
            tc.For_i(0, reps, 1) if reps > 1 else nullcontext(),
        ):
            # intermediate xf in DRAM, one tensor per m-chunk of 128:
            # [kc 3, kk 128, m 128, ri 2, c 32] bf16 (per-mch split lets
            # stage B start before all of stage A has finished)
            xf_ms = [dram.tile([3, 128, 128, 2, C_LOC], bf16, name=f"xfm{i}", tag=f"xfm{i}")
                     for i in range(3)]

            # F matrix resident in SBUF: [128, nchunk 3, (ri 2, m 384)]
            # col block ri*MPAD+m holds cos (ri=0) / -sin (ri=1) coefs
            f_tile = fpool.tile([NW, NCH, 2 * MPAD], f32r)
            nc.sync.dma_start(
                f_tile[:],
                f_d[:].rearrange("(a p) f -> p a f", p=NW),
            )

            # ---------------- Stage A: DFT over longitude ----------------
            for g in range(KCG if stage in ("AB", "A") else 0):
                # [128, (par 2, nc 3, 2048)]
                xt_tile = xtp.tile([NW, 2 * NCH * GW], f32r, tag="xt")
                nc.sync.dma_start(
                    xt_tile[:].rearrange("p (a q) -> p a q", a=2 * NCH),
                    xt_d[:, g * GW:(g + 1) * GW].rearrange(
                        "(a p) q -> p a q", p=NW),
                )
                kc_g = (g * 64) // 128          # which 128-row k chunk
                kk0 = (g * 64) % 128            # offset inside it
                for mch in range(3):
                    # ev holds both re and im interleaved: [m 128, (k 64, ri 2, c 32)]
                    ev = evp.tile([128, 2 * GW], bf16, tag="ev")
                    evv = ev[:].rearrange("m (k r c) -> m k r c", k=64, r=2)
                    for ri in range(2):
                        for ch in range(4):
                            ps = psA.tile([128, 512], f32, tag="psA")
                            for ncc in range(NCH):
                                col0 = ri * MPAD + mch * 128
                                rc0 = (ri * NCH + ncc) * GW + ch * 512
                                nc.tensor.matmul(
                                    ps[:],
                                    f_tile[:, ncc, col0:col0 + 128],
                                    xt_tile[:, rc0:rc0 + 512],
                                    start=(ncc == 0),
                                    stop=(ncc == NCH - 1),
                                )
                            nc.vector.tensor_copy(
                                evv[:, ch * 16:(ch + 1) * 16, ri, :],
                                ps[:].rearrange("m (k c) -> m k c", k=16))
                    nc.scalar.dma_start(
                        xf_ms[mch][kc_g, kk0:kk0 + 64, :, :, :]
                        .rearrange("k m r c -> m k r c"),
                        evv,
                    )

            # ---------------- Stage B: Legendre contraction ----------------
            # octets of 8 quads (32 m) share one xf load per kc
            n_oct = (NQ + 7) // 8
            for o in range(n_oct if stage in ("AB", "B") else 0):
                q0 = 8 * o
                q1 = min(q0 + 8, NQ)
                nq_o = q1 - q0
                mch = (4 * q0) // 128
                m0 = 4 * q0 - 128 * mch
                lts = []
                for kc in range(3):
                    lt = xflp.tile([128, nq_o * 256], bf16, tag="lt")
                    nc.sync.dma_start(
                        lt[:].rearrange("p (a r c) -> p a r c",
                                        a=4 * nq_o, r=2),
                        xf_ms[mch][kc, :, m0:m0 + 4 * nq_o, :, :],
                    )
                    lts.append(lt)
                for q in range(q0, q1):
                    L = _QL[q]
                    ql = q - q0
                    wt = wtp.tile([128, 3 * 4 * L], bf16, tag="wt")
                    off = _WQ_OFF[q]
                    nc.sync.dma_start(
                        wt[:].rearrange("p (kc f) -> p kc f", kc=3),
                        wq_d[off:off + 3 * 128 * 4 * L].rearrange(
                            "(kc p f) -> p kc f", kc=3, p=128),
                    )
                    ot = outp.tile([128, 2 * L], f32, tag="ot")
                    for ri in range(2):
                        ps = psB.tile([128, L], f32, tag="psB")
                        for mp in range(4):
                            for kc in range(3):
                                nc.tensor.matmul(
                                    ps[mp * 32:(mp + 1) * 32, :],
                                    lts[kc][:, (ql * 4 + mp) * 64 + ri * 32:
                                            (ql * 4 + mp) * 64 + ri * 32 + 32],
                                    wt[:, (kc * 4 + mp) * L:
                                       (kc * 4 + mp) * L + L],
                                    start=(kc == 0),
                                    stop=(kc == 2),
                                    tile_position=(0, mp * 32),
                                )
                        nc.vector.tensor_copy(
                            ot[:, ri * L:(ri + 1) * L], ps[:])
                    off = _OB_OFF[q]
                    nc.scalar.dma_start(
                        ob_d[off:off + 2 * 128 * L].rearrange(
                            "(r p l) -> p r l", r=2, p=128),
                        ot[:].rearrange("p (r l) -> p r l", r=2),
                    )

    nc.compile()
    return nc


def _dft_matrix():
    # folded: n = 0..360 only; cos for the re branch, -sin for the im branch
    n = np.arange(NLON // 2 + 1, dtype=np.float64)[:, None]
    m = np.arange(MMAX, dtype=np.float64)[None, :]
    ang = 2.0 * np.pi * n * m / NLON
    coef = 2.0 * np.pi / NLON
    F = np.zeros((NPAD, 2, MPAD), dtype=np.float32)
    F[:NLON // 2 + 1, 0, :MMAX] = (coef * np.cos(ang)).astype(np.float32)
    F[:NLON // 2 + 1, 1, :MMAX] = (-coef * np.sin(ang)).astype(np.float32)
    return F.reshape(NPAD, 2 * MPAD)


def _pack_weights(weights):
    # weights [MMAX, LMAX, NLAT] f32 -> per-quad blob bf16
    Wt = np.ascontiguousarray(np.transpose(weights, (0, 2, 1)))  # [m, k, l]
    blob = np.empty(WQ_TOTAL, dtype=BF16)
    for q in range(NQ):
        L = _QL[q]
        sub = np.zeros((4, KPAD, L), dtype=np.float32)
        m_hi = min(4 * q + 4, MMAX)
        sub[:m_hi - 4 * q, :NLAT, :] = Wt[4 * q:m_hi, :, 4 * q:]
        # [4 m', KPAD k, L] -> [kc 3, kk 128, m' 4, L]
        arr = sub.reshape(4, 3, 128, L).transpose(1, 2, 0, 3)
        blob[_WQ_OFF[q]:_WQ_OFF[q + 1]] = arr.astype(BF16).ravel()
    return blob


class _Runner:
    """jit(shard_map(bass_exec)) over the 8 cores, inputs stay addressable
    as sharded jax arrays so repeated timed executions skip host transfer."""

    def __init__(self, nc):
        import jax
        import concourse.mybir as mybir
        from jax.experimental.shard_map import shard_map
        from jax.sharding import Mesh, PartitionSpec, NamedSharding
        from concourse.bass2jax import (
            _bass_exec_p, install_neuronx_cc_hook, partition_id_tensor)

        install_neuronx_cc_hook()
        self.jax = jax
        self.nc = nc
        part_name = (nc.partition_id_tensor.name
                     if nc.partition_id_tensor else None)
        in_names, out_names, out_avals, zero_outs = [], [], [], []
        for alloc in nc.m.functions[0].allocations:
            if not isinstance(alloc, mybir.MemoryLocationSet):
                continue
            name = alloc.memorylocations[0].name
            if alloc.kind == "ExternalInput":
                if name != part_name:
                    in_names.append(name)
            elif alloc.kind == "ExternalOutput":
                shape = tuple(alloc.tensor_shape)
                dtype = mybir.dt.np(alloc.dtype)
                out_names.append(name)
                out_avals.append(jax.core.ShapedArray(shape, dtype))
                zero_outs.append(np.zeros(shape, dtype))
        self.in_names = list(in_names)
        self.out_names = out_names
        self.out_avals = out_avals
        self.zero_outs = zero_outs
        all_names = in_names + out_names
        if part_name is not None:
            all_names = all_names + [part_name]

        def _body(*args):
            operands = list(args)
            if part_name is not None:
                operands.append(partition_id_tensor())
            outs = _bass_exec_p.bind(
                *operands,
                out_avals=tuple(out_avals),
                in_names=tuple(all_names),
                out_names=tuple(out_names),
                lowering_input_output_aliases=(),
                sim_require_finite=True,
                sim_require_nnan=True,
                nc=nc,
            )
            return tuple(outs)

        devices = jax.devices()[:N_CORES]
        mesh = Mesh(np.asarray(devices), ("core",))
        spec = PartitionSpec("core")
        n_args = len(in_names) + len(out_names)
        self.sharding = NamedSharding(mesh, spec)
        self.fn = jax.jit(
            shard_map(_body, mesh=mesh,
                      in_specs=(spec,) * n_args,
                      out_specs=(spec,) * len(out_names),
                      check_rep=False),
            keep_unused=True,
        )

    def make_chain_fn(self, n_chain):
        """Build a jitted fn that executes the NEFF n_chain times serially
        (each iteration's first output feeds the next iteration's output
        placeholder, creating a data dependency that defeats CSE).  Used to
        measure per-execution device time above the fixed dispatch floor."""
        import jax
        from jax.experimental.shard_map import shard_map
        from jax.sharding import Mesh, PartitionSpec
        from concourse.bass2jax import _bass_exec_p, partition_id_tensor

        nc = self.nc
        part_name = (nc.partition_id_tensor.name
                     if nc.partition_id_tensor else None)
        all_names = list(self.in_names) + list(self.out_names)
        if part_name is not None:
            all_names = all_names + [part_name]
        out_avals = self.out_avals
        out_names = self.out_names

        def _body(*args):
            operands = list(args)
            if part_name is not None:
                operands.append(partition_id_tensor())
            last = None
            for _ in range(n_chain):
                # bass_exec carries BassEffect, so repeated identical calls
                # are neither CSE'd nor DCE'd; they serialize on the device
                # stream.
                last = _bass_exec_p.bind(
                    *operands,
                    out_avals=tuple(out_avals),
                    in_names=tuple(all_names),
                    out_names=tuple(out_names),
                    lowering_input_output_aliases=(),
                    sim_require_finite=True,
                    sim_require_nnan=True,
                    nc=nc,
                )
            return tuple(last)

        devices = self.jax.devices()[:N_CORES]
        mesh = Mesh(np.asarray(devices), ("core",))
        spec = PartitionSpec("core")
        n_args = len(self.in_names) + len(self.out_names)
        return jax.jit(
            shard_map(_body, mesh=mesh,
                      in_specs=(spec,) * n_args,
                      out_specs=(spec,) * len(self.out_names),
                      check_rep=False),
            keep_unused=True,
        )

    def device_args(self, in_maps):
        """Concat per-core inputs on axis 0 and put on the mesh."""
        jax = self.jax
        args = []
        for i, name in enumerate(self.in_names):
            cat = np.concatenate([m[name] for m in in_maps], axis=0)
            args.append(jax.device_put(cat, self.sharding))
        for z in self.zero_outs:
            cat = np.zeros((N_CORES * z.shape[0], *z.shape[1:]), z.dtype)
            args.append(jax.device_put(cat, self.sharding))
        return args

    def execute(self, args):
        outs = self.fn(*args)
        self.jax.block_until_ready(outs)
        return outs

    def run(self, in_maps):
        outs = self.execute(self.device_args(in_maps))
        results = []
        for c in range(N_CORES):
            r = {}
            for i, name in enumerate(self.out_names):
                full = np.asarray(outs[i])
                r[name] = full.reshape(N_CORES, *self.out_avals[i].shape)[c]
            results.append(r)
        return results


def get_runner(reps=1, stage="AB"):
    key = ("runner", reps, stage)
    if key not in _CACHE:
        _CACHE[key] = _Runner(_build_bass(reps, stage))
    if "F" not in _CACHE:
        _CACHE["F"] = _dft_matrix()
    return _CACHE[key]


def prepare_in_maps(x, weights):
    if "F" not in _CACHE:
        _CACHE["F"] = _dft_matrix()
    F = _CACHE["F"]
    x = np.asarray(x, dtype=np.float32)
    weights = np.asarray(weights, dtype=np.float32)
    wq = _pack_weights(weights)
    in_maps = []
    nh = NLON // 2  # 360
    for p in range(N_CORES):
        xs = x[0, p * C_LOC:(p + 1) * C_LOC]          # [32, 361, 720]
        xn = xs.transpose(2, 1, 0)                    # [720, 361, 32]
        xt = np.zeros((2, NPAD, KPAD, C_LOC), dtype=np.float32)
        # cos branch: x[0], x[n]+x[720-n] (n=1..359), x[360]
        xt[0, 0, :NLAT] = xn[0]
        xt[0, 1:nh, :NLAT] = xn[1:nh] + xn[:nh:-1]
        xt[0, nh, :NLAT] = xn[nh]
        # sin branch: x[n]-x[720-n] (n=1..359)
        xt[1, 1:nh, :NLAT] = xn[1:nh] - xn[:nh:-1]
        in_maps.append({
            "xt": np.ascontiguousarray(
                xt.reshape(2 * NPAD, KPAD * C_LOC)),
            "fm": F,
            "wq": wq,
        })
    return in_maps


def unpack_results(results):
    out_re = np.zeros((C, LMAX, MMAX), dtype=np.float32)
    out_im = np.zeros((C, LMAX, MMAX), dtype=np.float32)
    for p in range(N_CORES):
        ob = results[p]["ob"]
        c0 = p * C_LOC
        for q in range(NQ):
            L = _QL[q]
            arr = ob[_OB_OFF[q]:_OB_OFF[q + 1]].reshape(2, 4, C_LOC, L)
            # arr[ri, m', c, l'] -> out[c, l0+l', 4q+m']
            out_re[c0:c0 + C_LOC, 4 * q:, 4 * q:4 * q + 4] = \
                arr[0].transpose(1, 2, 0)
            out_im[c0:c0 + C_LOC, 4 * q:, 4 * q:4 * q + 4] = \
                arr[1].transpose(1, 2, 0)

    out = (out_re + 1j * out_im).astype(np.complex64)
    return out.reshape(1, C, LMAX, MMAX)


def kernel(x, weights):
    runner = get_runner()
    in_maps = prepare_in_maps(x, weights)
    results = runner.run(in_maps)
    return unpack_results(results)


# revision 21
# speedup vs baseline: 1.8952x; 1.3238x over previous
"""Distributed real SHT (spherical harmonic transform) for Trainium2.

Computes, for x [1, 256, 361, 720] f32 and weights [361, 360, 361] f32:
    xf = 2*pi * rfft(x, axis=-1, norm='forward')[..., :361]
    out_re = einsum('bckm,mlk->bclm', Re(xf), weights)
    out_im = einsum('bckm,mlk->bclm', Im(xf), weights)
    return complex64 [1, 256, 360, 361]

Sharding: channels (dim 1) across 8 NeuronCores, 32 channels each.

Symmetries exploited (both validated exactly against the reference arrays):
  * longitude fold: cos/sin DFT kernels are (anti)symmetric about n=360, so
    the host ships x[n]+/-x[720-n] and each DFT branch contracts only 361
    longitudes instead of 720 (halves stage-A matmul work).
  * latitude parity: P_l^m(-x) = (-1)^(l+m) P_l^m(x) on the symmetric
    Lobatto grid (bit-exact in the shipped weights), so the host also folds
    latitude pairs k/360-k into even/odd branches; stage B contracts 181
    latitudes per parity and ships only half the weight bytes.

Per-core pipeline:
  Stage A (DFT): matmuls xf[m, u] = sum_n F[n, ri, m] xt[n, u] over folded
    longitude chunks, u = (s 2, kfold 192, c 32) free layout, PSUM
    accumulated, cast to bf16, stored to a DRAM xf scratch split by m-chunk
    (so stage B can start early).
  Stage B (Legendre): bf16 matmuls per m-quad with 4x column tiling;
    for each (quad, m', re/im, parity): out[(m',c), l_par] accumulates
    2 chunks of 96 folded latitudes. The l < m output region is exactly
    zero (zero weights); the l < 4q region is restored host-side as zeros.
"""

import numpy as np
import ml_dtypes

NLAT = 361          # latitudes (k)
NLON = 720          # longitudes (n)
LMAX = 360          # output degree count (l = 0..359)
MMAX = 361          # rfft modes kept; m=360 output is all-zero (l<m)
C = 256
N_CORES = 8
C_LOC = C // N_CORES        # 32 channels per core
MPAD = 384                  # modes padded to 3*128
NPAD = 384                  # folded longitude (361 rows) padded to 3*128
NCH = 3                     # folded n split into 3 chunks of 128
KH = 180                    # latitude fold midpoint (k=180 self-paired)
KSP = 192                   # folded latitudes (181) padded to 2*96
NU = 2 * KSP                # free row-units: (s 2, kfold 192) = 384
FREEW = NU * C_LOC          # stage-A free width = 12288
KCG = 8                     # free width split into 8 groups of 48 units
GUN = NU // KCG             # 48 row-units per group
GW = GUN * C_LOC            # 1536 columns per group
NQ = LMAX // 4              # 90 m-quads (m = 4q + m', m' in 0..3)

XT_BF16 = True              # ship folded x / DFT matrix in bf16

BF16 = ml_dtypes.bfloat16

_QL = [LMAX - 4 * q for q in range(NQ)]
_QLH = [L // 2 for L in _QL]
_WQ_OFF = np.cumsum([0] + [4 * 96 * 4 * Lh for Lh in _QLH]).tolist()
_OB_OFF = np.cumsum([0] + [2 * 128 * L for L in _QL]).tolist()
WQ_TOTAL = _WQ_OFF[-1]
OB_TOTAL = _OB_OFF[-1]

_CACHE = {}


def _build_bass(reps=1, stage="AB"):
    import concourse.mybir as mybir
    import concourse.tile as tile
    from concourse import bacc
    from contextlib import nullcontext

    xdt = mybir.dt.bfloat16 if XT_BF16 else mybir.dt.float32r
    bf16 = mybir.dt.bfloat16
    f32 = mybir.dt.float32

    nc = bacc.Bacc("TRN2", target_bir_lowering=False, debug=False,
                   num_devices=N_CORES)

    # xt rows: (par 2, npad 384) folded longitudes; par 0 = cos branch
    # x[n]+x[720-n], par 1 = sin branch x[n]-x[720-n].
    # xt cols: (s 2, kfold 192, c 32) folded latitudes.
    xt_d = nc.dram_tensor("xt", [2 * NPAD, FREEW], xdt, kind="ExternalInput")
    f_d = nc.dram_tensor("fm", [NPAD, 2 * MPAD], xdt, kind="ExternalInput")
    wq_d = nc.dram_tensor("wq", [WQ_TOTAL], bf16, kind="ExternalInput")
    ob_d = nc.dram_tensor("ob", [OB_TOTAL], f32, kind="ExternalOutput")

    with tile.TileContext(nc) as tc:
        with (
            tc.tile_pool(name="dram", bufs=1, space="DRAM") as dram,
            tc.tile_pool(name="fpool", bufs=1) as fpool,
            tc.tile_pool(name="xtp", bufs=2) as xtp,
            tc.tile_pool(name="evict", bufs=3) as evp,
            tc.tile_pool(name="xfl", bufs=6) as xflp,
            tc.tile_pool(name="wt", bufs=3) as wtp,
            tc.tile_pool(name="outp", bufs=3) as outp,
            tc.tile_pool(name="psA", bufs=4, space="PSUM") as psA,
            tc.tile_pool(name="psB", bufs=4, space="PSUM") as psB,
            tc.For_i(0, reps, 1) if reps > 1 else nullcontext(),
        ):
            # xf scratch in DRAM, one tensor per m-chunk of 128:
            # [plane 4, row 96, m 128, ri 2, c 32] bf16; plane = (s, khalf)
            xf_ms = [dram.tile([4, 96, 128, 2, C_LOC], bf16,
                               name=f"xfm{i}", tag=f"xfm{i}")
                     for i in range(3)]

            # F matrix resident in SBUF: [128, nchunk 3, (ri 2, m 384)];
            # ri=0 cols hold cos coefs, ri=1 cols hold -sin coefs
            f_tile = fpool.tile([128, NCH, 2 * MPAD], xdt)
            nc.sync.dma_start(
                f_tile[:],
                f_d[:].rearrange("(a p) f -> p a f", p=128),
            )

            # ---------------- Stage A: DFT over folded longitude ----------
            for g in range(KCG if stage in ("AB", "A") else 0):
                # [128, (par 2, nc 3, 1536)]
                xt_tile = xtp.tile([128, 2 * NCH * GW], xdt, tag="xt")
                nc.sync.dma_start(
                    xt_tile[:].rearrange("p (a q) -> p a q", a=2 * NCH),
                    xt_d[:, g * GW:(g + 1) * GW].rearrange(
                        "(a p) q -> p a q", p=128),
                )
                plane = g // 2
                poff = 48 * (g % 2)
                for mch in range(3):
                    # both re and im interleaved: [m 128, (u 48, ri 2, c 32)]
                    ev = evp.tile([128, 2 * GW], bf16, tag="ev")
                    evv = ev[:].rearrange("m (u r c) -> m u r c", u=GUN, r=2)
                    for ri in range(2):
                        for ch in range(3):
                            ps = psA.tile([128, 512], f32, tag="psA")
                            for ncc in range(NCH):
                                col0 = ri * MPAD + mch * 128
                                rc0 = (ri * NCH + ncc) * GW + ch * 512
                                nc.tensor.matmul(
                                    ps[:],
                                    f_tile[:, ncc, col0:col0 + 128],
                                    xt_tile[:, rc0:rc0 + 512],
                                    start=(ncc == 0),
                                    stop=(ncc == NCH - 1),
                                )
                            nc.vector.tensor_copy(
                                evv[:, ch * 16:(ch + 1) * 16, ri, :],
                                ps[:].rearrange("m (u c) -> m u c", u=16))
                    nc.scalar.dma_start(
                        xf_ms[mch][plane, poff:poff + GUN, :, :, :]
                        .rearrange("u m r c -> m u r c"),
                        evv,
                    )

            # ---------------- Stage B: Legendre contraction ----------------
            # octets of 8 quads (32 m) share one xf load per plane
            n_oct = (NQ + 7) // 8
            for o in range(n_oct if stage in ("AB", "B") else 0):
                q0 = 8 * o
                q1 = min(q0 + 8, NQ)
                nm_o = 4 * (q1 - q0)
                mch = (4 * q0) // 128
                m0 = 4 * q0 - 128 * mch
                lts = []
                for ck in range(4):
                    lt = xflp.tile([96, nm_o * 64], bf16, tag="lt")
                    nc.sync.dma_start(
                        lt[:].rearrange("p (a r c) -> p a r c", a=nm_o, r=2),
                        xf_ms[mch][ck, :, m0:m0 + nm_o, :, :],
                    )
                    lts.append(lt)
                for q in range(q0, q1):
                    L = _QL[q]
                    Lh = _QLH[q]
                    ql = q - q0
                    # [96, (ck 4, m' 4, Lh)]
                    wt = wtp.tile([96, 16 * Lh], bf16, tag="wt")
                    off = _WQ_OFF[q]
                    nc.sync.dma_start(
                        wt[:].rearrange("p (a f) -> p a f", a=4),
                        wq_d[off:off + 4 * 96 * 4 * Lh].rearrange(
                            "(a p f) -> p a f", a=4, p=96),
                    )
                    ot = outp.tile([128, 2 * L], f32, tag="ot")
                    for ri in range(2):
                        ps = psB.tile([128, L], f32, tag="psB")
                        for mp in range(4):
                            col = (ql * 4 + mp) * 64 + ri * 32
                            for s in range(2):
                                for sub in range(2):
                                    ck = 2 * s + sub
                                    nc.tensor.matmul(
                                        ps[mp * 32:(mp + 1) * 32,
                                           s * Lh:(s + 1) * Lh],
                                        lts[ck][:, col:col + 32],
                                        wt[:, (ck * 4 + mp) * Lh:
                                           (ck * 4 + mp + 1) * Lh],
                                        start=(sub == 0),
                                        stop=(sub == 1),
                                        tile_position=(0, mp * 32),
                                    )
                        nc.vector.tensor_copy(
                            ot[:, ri * L:(ri + 1) * L], ps[:])
                    off = _OB_OFF[q]
                    nc.scalar.dma_start(
                        ob_d[off:off + 2 * 128 * L].rearrange(
                            "(r p l) -> p r l", r=2, p=128),
                        ot[:].rearrange("p (r l) -> p r l", r=2),
                    )

    nc.compile()
    return nc


def _dft_matrix():
    # folded longitude: n = 0..360; cos for re branch, -sin for im branch
    n = np.arange(NLON // 2 + 1, dtype=np.float64)[:, None]
    m = np.arange(MMAX, dtype=np.float64)[None, :]
    ang = 2.0 * np.pi * n * m / NLON
    coef = 2.0 * np.pi / NLON
    F = np.zeros((NPAD, 2, MPAD), dtype=np.float32)
    F[:NLON // 2 + 1, 0, :MMAX] = (coef * np.cos(ang)).astype(np.float32)
    F[:NLON // 2 + 1, 1, :MMAX] = (-coef * np.sin(ang)).astype(np.float32)
    F = F.reshape(NPAD, 2 * MPAD)
    return F.astype(BF16) if XT_BF16 else F


def _parity_ls(q, mp):
    """l-lists for quad q, mode m=4q+mp: index p means (l+m) % 2 == p."""
    m = 4 * q + mp
    l0 = 4 * q
    out = []
    for p in range(2):
        start = l0 if (l0 + m) % 2 == p else l0 + 1
        out.append(np.arange(start, LMAX, 2))
    return out


def _pack_weights(weights):
    # weights [MMAX, LMAX, NLAT] f32 -> folded-parity per-quad blob bf16:
    # per quad: [ck 4, 96, m' 4, Lh]; ck = 2*parity + sub96
    blob = np.empty(WQ_TOTAL, dtype=BF16)
    rev = np.arange(NLAT - 1, -1, -1)
    for q in range(NQ):
        Lh = _QLH[q]
        arr = np.zeros((4, 96, 4, Lh), dtype=np.float32)
        for mp in range(4):
            m = 4 * q + mp
            lls = _parity_ls(q, mp)
            for p in range(2):
                ls = lls[p]
                sign = 1.0 - 2.0 * p
                Wm = weights[m][ls]                      # [Lh, 361]
                Wf = np.zeros((Lh, KSP), dtype=np.float32)
                Wf[:, :KH] = 0.5 * (Wm[:, :KH] + sign * Wm[:, rev[:KH]])
                Wf[:, KH] = Wm[:, KH]
                arr[2 * p, :, mp, :] = Wf[:, :96].T
                arr[2 * p + 1, :, mp, :] = Wf[:, 96:].T
        blob[_WQ_OFF[q]:_WQ_OFF[q + 1]] = arr.astype(BF16).ravel()
    return blob


class _Runner:
    """jit(shard_map(bass_exec)) over the 8 cores; inputs stay resident as
    sharded jax arrays so repeated timed executions skip host transfer."""

    def __init__(self, nc):
        import jax
        import concourse.mybir as mybir
        from jax.experimental.shard_map import shard_map
        from jax.sharding import Mesh, PartitionSpec, NamedSharding
        from concourse.bass2jax import (
            _bass_exec_p, install_neuronx_cc_hook, partition_id_tensor)

        install_neuronx_cc_hook()
        self.jax = jax
        self.nc = nc
        part_name = (nc.partition_id_tensor.name
                     if nc.partition_id_tensor else None)
        in_names, out_names, out_avals, zero_outs = [], [], [], []
        for alloc in nc.m.functions[0].allocations:
            if not isinstance(alloc, mybir.MemoryLocationSet):
                continue
            name = alloc.memorylocations[0].name
            if alloc.kind == "ExternalInput":
                if name != part_name:
                    in_names.append(name)
            elif alloc.kind == "ExternalOutput":
                shape = tuple(alloc.tensor_shape)
                dtype = mybir.dt.np(alloc.dtype)
                out_names.append(name)
                out_avals.append(jax.core.ShapedArray(shape, dtype))
                zero_outs.append(np.zeros(shape, dtype))
        self.in_names = list(in_names)
        self.out_names = out_names
        self.out_avals = out_avals
        self.zero_outs = zero_outs
        all_names = in_names + out_names
        if part_name is not None:
            all_names = all_names + [part_name]

        def _body(*args):
            operands = list(args)
            if part_name is not None:
                operands.append(partition_id_tensor())
            outs = _bass_exec_p.bind(
                *operands,
                out_avals=tuple(out_avals),
                in_names=tuple(all_names),
                out_names=tuple(out_names),
                lowering_input_output_aliases=(),
                sim_require_finite=True,
                sim_require_nnan=True,
                nc=nc,
            )
            return tuple(outs)

        devices = jax.devices()[:N_CORES]
        mesh = Mesh(np.asarray(devices), ("core",))
        spec = PartitionSpec("core")
        n_args = len(in_names) + len(out_names)
        self.sharding = NamedSharding(mesh, spec)
        self.fn = jax.jit(
            shard_map(_body, mesh=mesh,
                      in_specs=(spec,) * n_args,
                      out_specs=(spec,) * len(out_names),
                      check_rep=False),
            keep_unused=True,
        )

    def device_args(self, in_maps):
        jax = self.jax
        args = []
        for name in self.in_names:
            cat = np.concatenate([m[name] for m in in_maps], axis=0)
            args.append(jax.device_put(cat, self.sharding))
        for z in self.zero_outs:
            cat = np.zeros((N_CORES * z.shape[0], *z.shape[1:]), z.dtype)
            args.append(jax.device_put(cat, self.sharding))
        return args

    def execute(self, args):
        outs = self.fn(*args)
        self.jax.block_until_ready(outs)
        return outs

    def run(self, in_maps):
        outs = self.execute(self.device_args(in_maps))
        results = []
        for c in range(N_CORES):
            r = {}
            for i, name in enumerate(self.out_names):
                full = np.asarray(outs[i])
                r[name] = full.reshape(N_CORES, *self.out_avals[i].shape)[c]
            results.append(r)
        return results


def get_runner(reps=1, stage="AB"):
    key = ("runner", reps, stage)
    if key not in _CACHE:
        _CACHE[key] = _Runner(_build_bass(reps, stage))
    return _CACHE[key]


def prepare_in_maps(x, weights):
    if "F" not in _CACHE:
        _CACHE["F"] = _dft_matrix()
    F = _CACHE["F"]
    x = np.asarray(x, dtype=np.float32)
    weights = np.asarray(weights, dtype=np.float32)
    wq = _pack_weights(weights)

    nh = NLON // 2  # 360
    in_maps = []
    for p in range(N_CORES):
        xs = x[0, p * C_LOC:(p + 1) * C_LOC]          # [32, 361, 720]
        xn = xs.transpose(2, 1, 0)                    # [720 n, 361 k, 32 c]
        # latitude fold: [720, s 2, 192, 32]
        xkf = np.zeros((NLON, 2, KSP, C_LOC), dtype=np.float32)
        for s in range(2):
            sign = 1.0 - 2.0 * s
            xkf[:, s, :KH] = xn[:, :KH] + sign * xn[:, NLAT - 1:KH:-1]
            xkf[:, s, KH] = xn[:, KH]
        # longitude fold: [par 2, npad 384, s 2, 192, 32]
        xt = np.zeros((2, NPAD, 2, KSP, C_LOC), dtype=np.float32)
        xt[0, 0] = xkf[0]
        xt[0, 1:nh] = xkf[1:nh] + xkf[:nh:-1]
        xt[0, nh] = xkf[nh]
        xt[1, 1:nh] = xkf[1:nh] - xkf[:nh:-1]
        xt = xt.reshape(2 * NPAD, FREEW)
        in_maps.append({
            "xt": xt.astype(BF16) if XT_BF16 else np.ascontiguousarray(xt),
            "fm": F,
            "wq": wq,
        })
    return in_maps


def unpack_results(results):
    out_re = np.zeros((C, LMAX, MMAX), dtype=np.float32)
    out_im = np.zeros((C, LMAX, MMAX), dtype=np.float32)
    for p in range(N_CORES):
        ob = results[p]["ob"]
        c0 = p * C_LOC
        for q in range(NQ):
            L = _QL[q]
            Lh = _QLH[q]
            arr = ob[_OB_OFF[q]:_OB_OFF[q + 1]].reshape(2, 4, C_LOC, L)
            for mp in range(4):
                m = 4 * q + mp
                lls = _parity_ls(q, mp)
                for s in range(2):
                    ls = lls[s]
                    out_re[c0:c0 + C_LOC, ls, m] = \
                        arr[0, mp, :, s * Lh:(s + 1) * Lh]
                    out_im[c0:c0 + C_LOC, ls, m] = \
                        arr[1, mp, :, s * Lh:(s + 1) * Lh]
    out = (out_re + 1j * out_im).astype(np.complex64)
    return out.reshape(1, C, LMAX, MMAX)


def kernel(x, weights):
    runner = get_runner()
    in_maps = prepare_in_maps(x, weights)
    results = runner.run(in_maps)
    return unpack_results(results)
